# revision 10
# baseline (speedup 1.0000x reference)
"""Trainium2 Bass kernel for DPPDynamicEmbedding (retrieval_knn).

Reference computation (per batch b, N=4096 points in [0,1]^2):
  placed    = (~action_mask) & ~(keepout | probe)                  [N] bool
  d2[i,j]   = |x_i|^2 + |x_j|^2 - 2 x_i.x_j                        [N,N]
  density_i = |{j : placed_j and d2[i,j] < R^2}| / 20              [N]
  proj      = [placed, density] @ W                                [N, 384]
  out       = split(proj, 3) -> (glimpse_key, glimpse_val, logit_key)

Strategy: data-parallel, 2 batches per core on 8 cores.  Points are
x-sorted on the host; i-block ib covers sorted ranks [128*ib, +128)
(block-major: block ib's outputs are rows [128*ib, +128) of the sorted
projection; host un-permutes).  The j-domain is the x-sorted placed
subset; block ib only scans the window [lo_ib, lo_ib+W_ib) of placed
points with x within +-R of the block's x-range (window union over all
16 batches keeps the program SPMD-shared).  This cuts pair-scan work
~34% vs scanning all placed j.

Per i-block:
  - PE computes psum[i, j] = -2 x_i.x_j + sq_j with K=8 fp16 matmuls
    (hi/lo splits keep ~1e-6 accuracy), 512-wide chunks cycling PE row
    groups 0/32/64/96 via tile_position (operands host-replicated at
    partition offsets 0/32/64/96).
  - The compare d2 < R^2  <=>  psum < thresh_i splits between ACT
    (Sign + accum -> S) and DVE (is_lt + accum -> C) at a per-block
    wa balancing the two engines' measured rates.  Blocks wider than
    the 1536-col psum tile put the overflow in a second tile, scanned
    by DVE into a separate accumulator.
  - Per 8-block chunk, DVE merges counts T = 0.5*(S + wa) + C (exact
    fp16 ints) into columns [pl, pl, T, T] of a staging tile; a PE
    transpose (identity matmul) flips it to [4k, 128] fp16 in PSUM,
    one ACT copy lands it in SBUF as the projection lhsT.  Fully
    on-chip; projection of chunk k overlaps counts of chunk k+1.
  - Projection is one K=4 fp16 matmul per 128 points (rhs rows hi/lo
    of [W0, W1/20], replicated at each 4-partition offset so lhsT/rhs
    partition offsets agree).  Outputs are written fp16 (host
    upcasts and un-permutes the x-sort).
"""

import numpy as np

import concourse.bass as bass
import concourse.mybir as mybir
import concourse.tile as tile
from concourse import bacc, bass_utils

R2 = 0.16
SCALE = 20.0
BIG = 32768.0          # pad sentinel; fp16-exact and >> R2
N_CORES = 8
WCAP = 1536            # scan psum tile width

F32 = mybir.dt.float32
F16 = mybir.dt.float16

# chunk boundaries (i-blocks per transpose); last chunks small to
# shrink the end-of-kernel tail.
CHUNKS = ((0, 8), (8, 16), (16, 24), (24, 28), (28, 32))
PROJ_DELAY = 2         # blocks between chunk end and its proj blocks


def _split16(v):
    hi = v.astype(np.float16)
    lo = (v - hi.astype(np.float32)).astype(np.float16)
    return hi, lo


def _wa_for(W):
    """ACT-side width: balances ACT (236ns + wa/1.2) vs DVE
    (127ns + wd/0.96), measured.  Overflow beyond WCAP goes to DVE."""
    Wm = min(W, WCAP)
    wa = int(round(0.5556 * Wm - 58.0))
    return max(0, min(Wm, wa))


def build_program(N, BPC, Mpad, LO, WID):
    C = N // 128
    assert len(LO) == C and len(WID) == C
    WA = [_wa_for(w) for w in WID]
    has_wide = [w > WCAP for w in WID]
    chunk_wide = [any(has_wide[c0:c1]) for c0, c1 in CHUNKS]

    nc = bacc.Bacc("TRN2", target_bir_lowering=False, debug=False,
                   num_devices=N_CORES)

    xr_d = nc.dram_tensor("xirhs", [BPC, 4, 8, N + Mpad], F16,
                          kind="ExternalInput")
    ax_d = nc.dram_tensor("aux", [BPC, 128, 3 * C], F32,
                          kind="ExternalInput")
    id_d = nc.dram_tensor("ident", [128, 128], F32, kind="ExternalInput")
    rw_d = nc.dram_tensor("rhsW", [32, 8 * 384], F16, kind="ExternalInput")
    pj_d = nc.dram_tensor("proj", [BPC, N, 384], F16, kind="ExternalOutput")

    with tile.TileContext(nc) as tc:
        with (
            tc.tile_pool(name="const", bufs=BPC) as cpool,
            tc.tile_pool(name="accp", bufs=BPC) as accp,
            tc.tile_pool(name="sp", bufs=2, space="PSUM") as spp,
            tc.tile_pool(name="pp", bufs=2, space="PSUM") as ppp,
            tc.tile_pool(name="scr_a", bufs=3) as scra,
            tc.tile_pool(name="scr_d", bufs=3) as scrd,
            tc.tile_pool(name="ct", bufs=2) as ctp,
            tc.tile_pool(name="outsb", bufs=8) as outp,
            tc.tile_pool(name="w", bufs=1) as wpool,
        ):
            dma_engs = (nc.sync, nc.gpsimd)
            load_engs = (nc.sync, nc.gpsimd, nc.scalar)

            xr, aux = [None] * BPC, [None] * BPC
            acc_a, acc_d, acc_w = [], [], []

            def load_batch(b):
                t = cpool.tile([128, N + Mpad], F16, tag="xr", name=f"xr{b}")
                for g in range(4):
                    load_engs[g % 3].dma_start(
                        t[32 * g:32 * g + 8, :], xr_d.ap()[b][g])
                xr[b] = t
                t = cpool.tile([128, 3 * C], F32, tag="ax", name=f"ax{b}")
                load_engs[b % 3].dma_start(t[:], ax_d.ap()[b])
                aux[b] = t

            load_batch(0)
            ident = wpool.tile([128, 128], F32, tag="id", name="ident")
            nc.gpsimd.dma_start(ident[:], id_d.ap())
            rhsW = wpool.tile([32, 8 * 384], F16, tag="rw", name="rhsW")
            nc.scalar.dma_start(rhsW[:], rw_d.ap())
            for b in range(BPC):
                acc_a.append(accp.tile([128, C], F32, tag="aa", name=f"aa{b}"))
                acc_d.append(accp.tile([128, C], F32, tag="ad", name=f"ad{b}"))
                t = accp.tile([128, C], F32, tag="aw", name=f"aw{b}")
                nc.gpsimd.memset(t[:], 0.0)
                acc_w.append(t)

            def counts_block(b, ib):
                W, lo, wa = WID[ib], LO[ib], WA[ib]
                Wm = min(W, WCAP)
                isl = slice(ib * 128, (ib + 1) * 128)
                sp = spp.tile([128, WCAP], F32, tag="sp", name=f"sp_{b}_{ib}")
                thb = aux[b][:, ib:ib + 1]
                off = 0
                g = ib  # rotate PE row groups across chunks and blocks
                while off < Wm:
                    w = min(512, Wm - off)
                    grp = 32 * (g % 4)
                    nc.tensor.matmul(
                        sp[:, off:off + w],
                        xr[b][grp:grp + 8, isl],
                        xr[b][grp:grp + 8, N + lo + off:N + lo + off + w],
                        start=True, stop=True,
                        tile_position=(grp, 0))
                    off += 512
                    g += 1
                spx = None
                if W > WCAP:
                    spx = spp.tile([128, WCAP], F32, tag="sp",
                                   name=f"spx_{b}_{ib}")
                    grp = 32 * (g % 4)
                    nc.tensor.matmul(
                        spx[:, :W - WCAP],
                        xr[b][grp:grp + 8, isl],
                        xr[b][grp:grp + 8, N + lo + WCAP:N + lo + W],
                        start=True, stop=True,
                        tile_position=(grp, 0))
                sa = scra.tile([128, 1024], F16, tag="sa", name=f"sa_{b}_{ib}")
                nc.scalar.activation(
                    sa[:, :wa], sp[:, :wa],
                    mybir.ActivationFunctionType.Sign,
                    bias=thb, scale=-1.0,
                    accum_out=acc_a[b][:, ib:ib + 1])
                sd = scrd.tile([128, 1024], F16, tag="sd", name=f"sd_{b}_{ib}")
                nc.vector.tensor_scalar(
                    sd[:, :Wm - wa], sp[:, wa:Wm],
                    thb, None,
                    op0=mybir.AluOpType.is_lt,
                    op1=mybir.AluOpType.add,
                    accum_out=acc_d[b][:, ib:ib + 1])
                if spx is not None:
                    sx = scrd.tile([128, 1024], F16, tag="sd",
                                   name=f"sx_{b}_{ib}")
                    nc.vector.tensor_scalar(
                        sx[:, :W - WCAP], spx[:, :W - WCAP],
                        thb, None,
                        op0=mybir.AluOpType.is_lt,
                        op1=mybir.AluOpType.add,
                        accum_out=acc_w[b][:, ib:ib + 1])

            lhsT = {}  # (b, chunk) -> sbuf [32, 128] tile of [pl,pl,T,T]

            def merge_chunk(b, k):
                c0, c1 = CHUNKS[k]
                n4 = 4 * (c1 - c0)
                ct = ctp.tile([128, 32], F32, tag="ct", name=f"ct_{b}_{k}")
                ctv = ct[:].rearrange("p (c f) -> p c f", f=4)
                csl = slice(C + c0, C + c1)
                ncol = c1 - c0
                # pl columns (aux cols [C, 2C))
                nc.vector.tensor_copy(ctv[:, :ncol, 0], aux[b][:, csl])
                nc.vector.tensor_copy(ctv[:, :ncol, 1], aux[b][:, csl])
                csl = slice(c0, c1)
                # T = 0.5*(S + waC) + C (+ wide overflow)
                tmp = accp.tile([128, 8], F32, tag="tmp", name=f"tm_{b}_{k}")
                nc.vector.tensor_tensor(
                    tmp[:, :ncol], acc_a[b][:, csl],
                    aux[b][:, 2 * C + c0:2 * C + c1],
                    op=mybir.AluOpType.add)
                if chunk_wide[k]:
                    nc.vector.tensor_tensor(
                        tmp[:, :ncol], tmp[:, :ncol], acc_w[b][:, csl],
                        op=mybir.AluOpType.add)
                nc.vector.scalar_tensor_tensor(
                    ctv[:, :ncol, 2], tmp[:, :ncol], 0.5, acc_d[b][:, csl],
                    op0=mybir.AluOpType.mult, op1=mybir.AluOpType.add)
                nc.vector.tensor_copy(ctv[:, :ncol, 3], ctv[:, :ncol, 2])
                # transpose -> [4k, 128] f32 psum (borrow a scan tile) -> SBUF
                tps = spp.tile([128, WCAP], F32, tag="sp", name=f"tp_{b}_{k}")
                nc.tensor.transpose(tps[:n4, :128], ct[:, :n4], ident[:])
                lt = ctp.tile([32, 128], F16, tag="lt", name=f"lt_{b}_{k}")
                nc.scalar.copy(lt[:n4, :], tps[:n4, :128])
                lhsT[(b, k)] = lt

            osb_cur = [None]

            def proj_block(b, k, ib):
                c0, c1 = CHUNKS[k]
                q = ib - c0
                kk = 4 * (c1 - c0)
                po = ppp.tile([128, 512], F32, tag="po", name=f"po_{b}_{ib}")
                # lhsT is the whole transposed chunk (32-aligned partition
                # start); rhs variant q is zero outside rows 4q..4q+4, so
                # the K-dim contraction picks out block q's [pl,pl,T,T].
                nc.tensor.matmul(
                    po[:, :384],
                    lhsT[(b, k)][:kk, :],
                    rhsW[:kk, 384 * q:384 * q + 384], start=True, stop=True,
                    tile_position=(0, 0))
                half = ib % 2
                if half == 0:
                    osb_cur[0] = outp.tile([128, 768], F16, tag="osb",
                                           name=f"osb_{b}_{ib}")
                osb = osb_cur[0]
                dst = osb[:].rearrange("p (s f) -> p s f", s=2)[:, half, :]
                if half == 0:
                    nc.vector.tensor_copy(dst, po[:, :384])
                else:
                    nc.scalar.copy(dst, po[:, :384])
                    src_ = osb[:].rearrange("p (s f) -> p s f", s=2)
                    ddst = pj_d.ap()[b, (ib - 1) * 128:(ib + 1) * 128, :] \
                        .rearrange("(s p) f -> p s f", p=128)
                    dma_engs[(ib // 2) % 2].dma_start(ddst, src_)

            # ---- schedule ----
            pending = []
            ready_at = {}
            for b in range(BPC):
                for ib in range(32):
                    counts_block(b, ib)
                    if b + 1 < BPC and ib == 1:
                        load_batch(b + 1)
                    here = b * 32 + ib
                    for _ in range(3):
                        if pending and ready_at[pending[0]] <= here:
                            pb, pk, pib = pending.pop(0)
                            proj_block(pb, pk, pib)
                    for k, (c0, c1) in enumerate(CHUNKS):
                        if ib == c1 - 1:
                            merge_chunk(b, k)
                            for j in range(c0, c1):
                                pending.append((b, k, j))
                                ready_at[(b, k, j)] = here + PROJ_DELAY
            for pb, pk, pib in pending:
                proj_block(pb, pk, pib)
    nc.compile()
    return nc


def _windows(locs, placed):
    """Union (over batches) per-block j-windows into x-sorted placed."""
    B, N, _ = locs.shape
    C = N // 128
    R = np.float32(np.sqrt(R2))
    lo = np.full(C, 1 << 30)
    hi = np.zeros(C, dtype=np.int64)
    for b in range(B):
        x = locs[b, :, 0]
        xs = np.sort(x)
        px = np.sort(x[placed[b]])
        l = np.searchsorted(px, xs[::128] - R, side="left")
        h = np.searchsorted(px, xs[127::128] + R, side="right")
        np.minimum(lo, l, out=lo)
        np.maximum(hi, h, out=hi)
    wid = hi - lo
    return lo.astype(int).tolist(), wid.astype(int).tolist()


def prep_core_inputs(action_mask, keepout, probe, locs, W, Mpad, LO, WID):
    BPC, N, _ = locs.shape
    C = N // 128
    WA_tot = [_wa_for(w) for w in WID]

    placed = (~action_mask) & ~(keepout | probe)
    x = locs.astype(np.float32)
    sq = (x ** 2).sum(-1)

    xirhs = np.zeros((BPC, 8, N + Mpad), np.float16)
    aux = np.zeros((BPC, 128, 3 * C), np.float32)
    orders = []

    for b in range(BPC):
        order = np.argsort(x[b, :, 0], kind="stable")
        orders.append(order)
        xs = x[b, order]
        sqs = sq[b, order]
        pls = placed[b, order].astype(np.float32)

        x0h, x0l = _split16(xs[:, 0])
        x1h, x1l = _split16(xs[:, 1])
        xirhs[b, 0, :N] = x0h
        xirhs[b, 1, :N] = x0h
        xirhs[b, 2, :N] = x0l
        xirhs[b, 3, :N] = x1h
        xirhs[b, 4, :N] = x1h
        xirhs[b, 5, :N] = x1l
        xirhs[b, 6, :N] = 1.0
        xirhs[b, 7, :N] = 1.0

        pidx = np.nonzero(placed[b])[0]
        psort = pidx[np.argsort(x[b, pidx, 0], kind="stable")]
        np_ = len(psort)
        assert np_ <= Mpad
        j0h, j0l = _split16(-2.0 * x[b, psort, 0])
        j1h, j1l = _split16(-2.0 * x[b, psort, 1])
        sqh, sql = _split16(sq[b, psort])
        r = xirhs[b, :, N:]
        r[6, :] = BIG
        r[0, :np_] = j0h
        r[1, :np_] = j0l
        r[2, :np_] = j0h
        r[3, :np_] = j1h
        r[4, :np_] = j1l
        r[5, :np_] = j1h
        r[6, :np_] = sqh
        r[7, :np_] = sql

        aux[b, :, 0:C] = (R2 - sqs).reshape(C, 128).T
        aux[b, :, C:2 * C] = pls.reshape(C, 128).T
        aux[b, :, 2 * C:] = np.asarray(WA_tot, np.float32)[None, :]

    W = W.astype(np.float32)
    rhsW = np.zeros((32, 8 * 384), np.float16)
    for q in range(8):
        for r_, v in enumerate([W[0], W[1] / SCALE]):
            h, lo_ = _split16(v)
            rhsW[4 * q + 2 * r_, 384 * q:384 * (q + 1)] = h
            rhsW[4 * q + 2 * r_ + 1, 384 * q:384 * (q + 1)] = lo_

    ident = np.eye(128, dtype=np.float32)
    rep4 = lambda a: np.repeat(a[:, None], 4, axis=1)
    return {
        "xirhs": rep4(xirhs), "aux": aux, "rhsW": rhsW, "ident": ident,
    }, orders


_PROGRAM_CACHE = {}


def kernel(action_mask, keepout, probe, locs, W, _trace=False, _tmpdir=None):
    action_mask = np.asarray(action_mask)
    keepout = np.asarray(keepout)
    probe = np.asarray(probe)
    locs = np.asarray(locs, dtype=np.float32)
    W = np.asarray(W, dtype=np.float32)

    B, N = action_mask.shape
    BPC = B // N_CORES

    placed = (~action_mask) & ~(keepout | probe)
    max_placed = int(placed.sum(1).max())
    Mpad = ((max_placed + 63) // 64) * 64
    LO, WID = _windows(locs, placed)

    key = (N, BPC, Mpad, tuple(LO), tuple(WID))
    if key not in _PROGRAM_CACHE:
        _PROGRAM_CACHE[key] = build_program(N, BPC, Mpad, LO, WID)
    nc = _PROGRAM_CACHE[key]

    in_maps = []
    orders = []
    for c in range(N_CORES):
        s = slice(c * BPC, (c + 1) * BPC)
        m, o = prep_core_inputs(
            action_mask[s], keepout[s], probe[s], locs[s], W, Mpad, LO, WID)
        in_maps.append(m)
        orders.extend(o)

    res = bass_utils.run_bass_kernel_spmd(
        nc, in_maps, core_ids=list(range(N_CORES)),
        trace=_trace, tmpdir=_tmpdir)

    proj_s = np.concatenate(
        [res.results[c]["proj"] for c in range(N_CORES)], 0)
    proj = np.empty((B, N, 384), np.float32)
    for b in range(B):
        proj[b, orders[b]] = proj_s[b].astype(np.float32)
    out = (np.ascontiguousarray(proj[:, :, :128]),
           np.ascontiguousarray(proj[:, :, 128:256]),
           np.ascontiguousarray(proj[:, :, 256:384]))
    if _trace:
        return out, res
    return out


# revision 11
# speedup vs baseline: 1.1328x; 1.1328x over previous
"""Trainium2 Bass kernel for DPPDynamicEmbedding (retrieval_knn).

Reference computation (per batch b, N=4096 points in [0,1]^2):
  placed    = (~action_mask) & ~(keepout | probe)                  [N] bool
  d2[i,j]   = |x_i|^2 + |x_j|^2 - 2 x_i.x_j                        [N,N]
  density_i = |{j : placed_j and d2[i,j] < R^2}| / 20              [N]
  proj      = [placed, density] @ W                                [N, 384]
  out       = split(proj, 3) -> (glimpse_key, glimpse_val, logit_key)

Strategy: data-parallel, 2 batches per core on 8 cores.  Points are
x-sorted on the host; i-block ib covers sorted ranks [128*ib, +128)
(block-major: block ib's outputs are rows [128*ib, +128) of the sorted
projection; host un-permutes).  The j-domain is the x-sorted placed
subset; block ib only scans the window [lo_ib, lo_ib+W_ib) of placed
points with x within +-R of the block's x-range (window union over all
16 batches keeps the program SPMD-shared).  This cuts pair-scan work
~34% vs scanning all placed j.

Per i-block:
  - PE computes psum[i, j] = -2 x_i.x_j + sq_j with K=8 fp16 matmuls
    (hi/lo splits keep ~1e-6 accuracy), 512-wide chunks cycling PE row
    groups 0/32/64/96 via tile_position (operands host-replicated at
    partition offsets 0/32/64/96).
  - The compare d2 < R^2  <=>  psum < thresh_i splits between ACT
    (Sign + accum -> S) and DVE (is_lt + accum -> C) at a per-block
    wa balancing the two engines' measured rates.  Blocks wider than
    the 1536-col psum tile put the overflow in a second tile, scanned
    by DVE into a separate accumulator.
  - Per 8-block chunk, DVE merges counts T = 0.5*(S + wa) + C (exact
    fp16 ints) into columns [pl, pl, T, T] of a staging tile; a PE
    transpose (identity matmul) flips it to [4k, 128] fp16 in PSUM,
    one ACT copy lands it in SBUF as the projection lhsT.  Fully
    on-chip; projection of chunk k overlaps counts of chunk k+1.
  - Projection is one K=4 fp16 matmul per 128 points (rhs rows hi/lo
    of [W0, W1/20], replicated at each 4-partition offset so lhsT/rhs
    partition offsets agree).  Outputs are written fp16 (host
    upcasts and un-permutes the x-sort).
"""

import numpy as np

import concourse.bass as bass
import concourse.mybir as mybir
import concourse.tile as tile
from concourse import bacc, bass_utils

R2 = 0.16
SCALE = 20.0
BIG = 32768.0          # pad sentinel; fp16-exact and >> R2
N_CORES = 8
WCAP = 1536            # scan psum tile width

F32 = mybir.dt.float32
F16 = mybir.dt.float16

# chunk boundaries (i-blocks per transpose); last chunks small to
# shrink the end-of-kernel tail.
CHUNKS = ((0, 8), (8, 16), (16, 24), (24, 28), (28, 32))
PROJ_DELAY = 2         # blocks between chunk end and its proj blocks


def _split16(v):
    hi = v.astype(np.float16)
    lo = (v - hi.astype(np.float32)).astype(np.float16)
    return hi, lo


def _wa_for(W):
    """ACT-side width: balances ACT (236ns + wa/1.2) vs DVE
    (127ns + wd/0.96), measured.  Overflow beyond WCAP goes to DVE."""
    Wm = min(W, WCAP)
    wa = int(round(0.5556 * Wm - 58.0))
    return max(0, min(Wm, wa))


def build_program(N, BPC, Mpad, LO, WID):
    C = N // 128
    assert len(LO) == C and len(WID) == C
    WA = [_wa_for(w) for w in WID]
    has_wide = [w > WCAP for w in WID]
    chunk_wide = [any(has_wide[c0:c1]) for c0, c1 in CHUNKS]

    nc = bacc.Bacc("TRN2", target_bir_lowering=False, debug=False,
                   num_devices=N_CORES)

    xr_d = nc.dram_tensor("xirhs", [BPC, 4, 8, N + Mpad], F16,
                          kind="ExternalInput")
    ax_d = nc.dram_tensor("aux", [BPC, 128, 3 * C], F32,
                          kind="ExternalInput")
    id_d = nc.dram_tensor("ident", [128, 128], F32, kind="ExternalInput")
    rw_d = nc.dram_tensor("rhsW", [16, 8 * 384], F16, kind="ExternalInput")
    pj_d = nc.dram_tensor("proj", [BPC, N, 384], F16, kind="ExternalOutput")

    with tile.TileContext(nc) as tc:
        with (
            tc.tile_pool(name="const", bufs=BPC) as cpool,
            tc.tile_pool(name="accp", bufs=BPC) as accp,
            tc.tile_pool(name="sp", bufs=2, space="PSUM") as spp,
            tc.tile_pool(name="pp", bufs=2, space="PSUM") as ppp,
            tc.tile_pool(name="scr_a", bufs=3) as scra,
            tc.tile_pool(name="scr_d", bufs=3) as scrd,
            tc.tile_pool(name="ct", bufs=2) as ctp,
            tc.tile_pool(name="outsb", bufs=8) as outp,
            tc.tile_pool(name="w", bufs=1) as wpool,
        ):
            dma_engs = (nc.sync, nc.gpsimd)
            load_engs = (nc.sync, nc.gpsimd, nc.scalar)

            xr, aux = [None] * BPC, [None] * BPC
            acc_a, acc_d, acc_w = [], [], []

            def load_batch(b):
                t = cpool.tile([128, N + Mpad], F16, tag="xr", name=f"xr{b}")
                for g in range(4):
                    load_engs[g % 3].dma_start(
                        t[32 * g:32 * g + 8, :], xr_d.ap()[b][g])
                xr[b] = t
                t = cpool.tile([128, 3 * C], F32, tag="ax", name=f"ax{b}")
                load_engs[b % 3].dma_start(t[:], ax_d.ap()[b])
                aux[b] = t

            load_batch(0)
            ident = wpool.tile([128, 128], F32, tag="id", name="ident")
            nc.gpsimd.dma_start(ident[:], id_d.ap())
            rhsW = wpool.tile([16, 8 * 384], F16, tag="rw", name="rhsW")
            nc.scalar.dma_start(rhsW[:], rw_d.ap())
            for b in range(BPC):
                acc_a.append(accp.tile([128, C], F32, tag="aa", name=f"aa{b}"))
                acc_d.append(accp.tile([128, C], F32, tag="ad", name=f"ad{b}"))
                t = accp.tile([128, C], F32, tag="aw", name=f"aw{b}")
                nc.gpsimd.memset(t[:], 0.0)
                acc_w.append(t)

            def counts_block(b, ib):
                W, lo, wa = WID[ib], LO[ib], WA[ib]
                Wm = min(W, WCAP)
                isl = slice(ib * 128, (ib + 1) * 128)
                sp = spp.tile([128, WCAP], F32, tag="sp", name=f"sp_{b}_{ib}")
                thb = aux[b][:, ib:ib + 1]
                off = 0
                g = ib  # rotate PE row groups across chunks and blocks
                while off < Wm:
                    w = min(512, Wm - off)
                    grp = 32 * (g % 4)
                    nc.tensor.matmul(
                        sp[:, off:off + w],
                        xr[b][grp:grp + 8, isl],
                        xr[b][grp:grp + 8, N + lo + off:N + lo + off + w],
                        start=True, stop=True,
                        tile_position=(grp, 0))
                    off += 512
                    g += 1
                spx = None
                if W > WCAP:
                    spx = spp.tile([128, WCAP], F32, tag="sp",
                                   name=f"spx_{b}_{ib}")
                    grp = 32 * (g % 4)
                    nc.tensor.matmul(
                        spx[:, :W - WCAP],
                        xr[b][grp:grp + 8, isl],
                        xr[b][grp:grp + 8, N + lo + WCAP:N + lo + W],
                        start=True, stop=True,
                        tile_position=(grp, 0))
                sa = scra.tile([128, 1024], F16, tag="sa", name=f"sa_{b}_{ib}")
                nc.scalar.activation(
                    sa[:, :wa], sp[:, :wa],
                    mybir.ActivationFunctionType.Sign,
                    bias=thb, scale=-1.0,
                    accum_out=acc_a[b][:, ib:ib + 1])
                sd = scrd.tile([128, 1024], F16, tag="sd", name=f"sd_{b}_{ib}")
                nc.vector.tensor_scalar(
                    sd[:, :Wm - wa], sp[:, wa:Wm],
                    thb, None,
                    op0=mybir.AluOpType.is_lt,
                    op1=mybir.AluOpType.add,
                    accum_out=acc_d[b][:, ib:ib + 1])
                if spx is not None:
                    sx = scrd.tile([128, 1024], F16, tag="sd",
                                   name=f"sx_{b}_{ib}")
                    nc.vector.tensor_scalar(
                        sx[:, :W - WCAP], spx[:, :W - WCAP],
                        thb, None,
                        op0=mybir.AluOpType.is_lt,
                        op1=mybir.AluOpType.add,
                        accum_out=acc_w[b][:, ib:ib + 1])

            lhsT = {}   # (b, chunk) -> sbuf [16, 128] tile of [pl, T] rows
            cts = {}    # (b, chunk) -> staging tile

            def stage_chunk(b, k):
                """Copy pl columns early (no dependence on counts)."""
                c0, c1 = CHUNKS[k]
                ncol = c1 - c0
                ct = ctp.tile([128, 16], F32, tag="ct", name=f"ct_{b}_{k}")
                ctv = ct[:].rearrange("p (c f) -> p c f", f=2)
                nc.vector.tensor_copy(ctv[:, :ncol, 0],
                                      aux[b][:, C + c0:C + c1])
                cts[(b, k)] = ct

            def merge_chunk(b, k):
                c0, c1 = CHUNKS[k]
                n2 = 2 * (c1 - c0)
                ncol = c1 - c0
                ct = cts.pop((b, k))
                ctv = ct[:].rearrange("p (c f) -> p c f", f=2)
                csl = slice(c0, c1)
                # T = 0.5*(S + waC) + C (+ wide overflow), exact fp32
                tmp = accp.tile([128, 8], F32, tag="tmp", name=f"tm_{b}_{k}")
                nc.vector.tensor_tensor(
                    tmp[:, :ncol], acc_a[b][:, csl],
                    aux[b][:, 2 * C + c0:2 * C + c1],
                    op=mybir.AluOpType.add)
                if chunk_wide[k]:
                    nc.vector.tensor_tensor(
                        tmp[:, :ncol], tmp[:, :ncol], acc_w[b][:, csl],
                        op=mybir.AluOpType.add)
                nc.vector.scalar_tensor_tensor(
                    ctv[:, :ncol, 1], tmp[:, :ncol], 0.5, acc_d[b][:, csl],
                    op0=mybir.AluOpType.mult, op1=mybir.AluOpType.add)
                # transpose -> [2k, 128] f32 psum (borrow a proj tile) -> SBUF
                tps = ppp.tile([128, 512], F32, tag="po", name=f"tp_{b}_{k}")
                nc.tensor.transpose(tps[:n2, :128], ct[:, :n2], ident[:])
                lt = ctp.tile([16, 128], F16, tag="lt", name=f"lt_{b}_{k}")
                nc.scalar.copy(lt[:n2, :], tps[:n2, :128])
                lhsT[(b, k)] = lt

            osb_cur = [None]

            def proj_block(b, k, ib):
                c0, c1 = CHUNKS[k]
                q = ib - c0
                kk = 2 * (c1 - c0)
                po = ppp.tile([128, 512], F32, tag="po", name=f"po_{b}_{ib}")
                # lhsT is the whole transposed chunk (32-aligned partition
                # start); rhs variant q is zero outside rows 2q..2q+2, so
                # the K-dim contraction picks out block q's [pl, T].
                nc.tensor.matmul(
                    po[:, :384],
                    lhsT[(b, k)][:kk, :],
                    rhsW[:kk, 384 * q:384 * q + 384], start=True, stop=True,
                    tile_position=(0, 0))
                half = ib % 2
                if half == 0:
                    osb_cur[0] = outp.tile([128, 768], F16, tag="osb",
                                           name=f"osb_{b}_{ib}")
                osb = osb_cur[0]
                dst = osb[:].rearrange("p (s f) -> p s f", s=2)[:, half, :]
                if half == 0:
                    nc.vector.tensor_copy(dst, po[:, :384])
                else:
                    nc.scalar.copy(dst, po[:, :384])
                    src_ = osb[:].rearrange("p (s f) -> p s f", s=2)
                    ddst = pj_d.ap()[b, (ib - 1) * 128:(ib + 1) * 128, :] \
                        .rearrange("(s p) f -> p s f", p=128)
                    dma_engs[(ib // 2) % 2].dma_start(ddst, src_)

            # ---- schedule ----
            pending = []
            ready_at = {}
            for b in range(BPC):
                for ib in range(32):
                    for k, (c0, c1) in enumerate(CHUNKS):
                        if ib == c0:
                            stage_chunk(b, k)
                    counts_block(b, ib)
                    if b + 1 < BPC and ib == 1:
                        load_batch(b + 1)
                    here = b * 32 + ib
                    for _ in range(3):
                        if pending and ready_at[pending[0]] <= here:
                            pb, pk, pib = pending.pop(0)
                            proj_block(pb, pk, pib)
                    for k, (c0, c1) in enumerate(CHUNKS):
                        if ib == c1 - 1:
                            merge_chunk(b, k)
                            for j in range(c0, c1):
                                pending.append((b, k, j))
                                ready_at[(b, k, j)] = here + PROJ_DELAY
            for pb, pk, pib in pending:
                proj_block(pb, pk, pib)
    nc.compile()
    return nc


def _windows(locs, placed):
    """Union (over batches) per-block j-windows into x-sorted placed."""
    B, N, _ = locs.shape
    C = N // 128
    R = np.float32(np.sqrt(R2))
    lo = np.full(C, 1 << 30)
    hi = np.zeros(C, dtype=np.int64)
    for b in range(B):
        x = locs[b, :, 0]
        xs = np.sort(x)
        px = np.sort(x[placed[b]])
        l = np.searchsorted(px, xs[::128] - R, side="left")
        h = np.searchsorted(px, xs[127::128] + R, side="right")
        np.minimum(lo, l, out=lo)
        np.maximum(hi, h, out=hi)
    wid = hi - lo
    return lo.astype(int).tolist(), wid.astype(int).tolist()


def prep_core_inputs(action_mask, keepout, probe, locs, W, Mpad, LO, WID):
    BPC, N, _ = locs.shape
    C = N // 128
    WA_tot = [_wa_for(w) for w in WID]

    placed = (~action_mask) & ~(keepout | probe)
    x = locs.astype(np.float32)
    sq = (x ** 2).sum(-1)

    xirhs = np.zeros((BPC, 8, N + Mpad), np.float16)
    aux = np.zeros((BPC, 128, 3 * C), np.float32)
    orders = []

    for b in range(BPC):
        order = np.argsort(x[b, :, 0], kind="stable")
        orders.append(order)
        xs = x[b, order]
        sqs = sq[b, order]
        pls = placed[b, order].astype(np.float32)

        x0h, x0l = _split16(xs[:, 0])
        x1h, x1l = _split16(xs[:, 1])
        xirhs[b, 0, :N] = x0h
        xirhs[b, 1, :N] = x0h
        xirhs[b, 2, :N] = x0l
        xirhs[b, 3, :N] = x1h
        xirhs[b, 4, :N] = x1h
        xirhs[b, 5, :N] = x1l
        xirhs[b, 6, :N] = 1.0
        xirhs[b, 7, :N] = 1.0

        pidx = np.nonzero(placed[b])[0]
        psort = pidx[np.argsort(x[b, pidx, 0], kind="stable")]
        np_ = len(psort)
        assert np_ <= Mpad
        j0h, j0l = _split16(-2.0 * x[b, psort, 0])
        j1h, j1l = _split16(-2.0 * x[b, psort, 1])
        sqh, sql = _split16(sq[b, psort])
        r = xirhs[b, :, N:]
        r[6, :] = BIG
        r[0, :np_] = j0h
        r[1, :np_] = j0l
        r[2, :np_] = j0h
        r[3, :np_] = j1h
        r[4, :np_] = j1l
        r[5, :np_] = j1h
        r[6, :np_] = sqh
        r[7, :np_] = sql

        aux[b, :, 0:C] = (R2 - sqs).reshape(C, 128).T
        aux[b, :, C:2 * C] = pls.reshape(C, 128).T
        aux[b, :, 2 * C:] = np.asarray(WA_tot, np.float32)[None, :]

    W = W.astype(np.float32)
    rhsW = np.zeros((16, 8 * 384), np.float16)
    for q in range(8):
        rhsW[2 * q, 384 * q:384 * (q + 1)] = W[0].astype(np.float16)
        rhsW[2 * q + 1, 384 * q:384 * (q + 1)] = (W[1] / SCALE).astype(
            np.float16)

    ident = np.eye(128, dtype=np.float32)
    rep4 = lambda a: np.repeat(a[:, None], 4, axis=1)
    return {
        "xirhs": rep4(xirhs), "aux": aux, "rhsW": rhsW, "ident": ident,
    }, orders


_PROGRAM_CACHE = {}


def kernel(action_mask, keepout, probe, locs, W, _trace=False, _tmpdir=None):
    action_mask = np.asarray(action_mask)
    keepout = np.asarray(keepout)
    probe = np.asarray(probe)
    locs = np.asarray(locs, dtype=np.float32)
    W = np.asarray(W, dtype=np.float32)

    B, N = action_mask.shape
    BPC = B // N_CORES

    placed = (~action_mask) & ~(keepout | probe)
    max_placed = int(placed.sum(1).max())
    Mpad = ((max_placed + 63) // 64) * 64
    LO, WID = _windows(locs, placed)

    key = (N, BPC, Mpad, tuple(LO), tuple(WID))
    if key not in _PROGRAM_CACHE:
        _PROGRAM_CACHE[key] = build_program(N, BPC, Mpad, LO, WID)
    nc = _PROGRAM_CACHE[key]

    in_maps = []
    orders = []
    for c in range(N_CORES):
        s = slice(c * BPC, (c + 1) * BPC)
        m, o = prep_core_inputs(
            action_mask[s], keepout[s], probe[s], locs[s], W, Mpad, LO, WID)
        in_maps.append(m)
        orders.extend(o)

    res = bass_utils.run_bass_kernel_spmd(
        nc, in_maps, core_ids=list(range(N_CORES)),
        trace=_trace, tmpdir=_tmpdir)

    proj_s = np.concatenate(
        [res.results[c]["proj"] for c in range(N_CORES)], 0)
    proj = np.empty((B, N, 384), np.float32)
    for b in range(B):
        proj[b, orders[b]] = proj_s[b].astype(np.float32)
    out = (np.ascontiguousarray(proj[:, :, :128]),
           np.ascontiguousarray(proj[:, :, 128:256]),
           np.ascontiguousarray(proj[:, :, 256:384]))
    if _trace:
        return out, res
    return out


# revision 12
# speedup vs baseline: 1.2335x; 1.0889x over previous
"""Trainium2 Bass kernel for DPPDynamicEmbedding (retrieval_knn).

Reference computation (per batch b, N=4096 points in [0,1]^2):
  placed    = (~action_mask) & ~(keepout | probe)                  [N] bool
  d2[i,j]   = |x_i|^2 + |x_j|^2 - 2 x_i.x_j                        [N,N]
  density_i = |{j : placed_j and d2[i,j] < R^2}| / 20              [N]
  proj      = [placed, density] @ W                                [N, 384]
  out       = split(proj, 3) -> (glimpse_key, glimpse_val, logit_key)

Strategy: data-parallel, 2 batches per core on 8 cores.  Points are
x-sorted on the host; i-block ib covers sorted ranks [128*ib, +128)
(block-major: block ib's outputs are rows [128*ib, +128) of the sorted
projection; host un-permutes).  The j-domain is the x-sorted placed
subset; block ib only scans the window [lo_ib, lo_ib+W_ib) of placed
points with x within +-R of the block's x-range (window union over all
16 batches keeps the program SPMD-shared).  This cuts pair-scan work
~34% vs scanning all placed j.

Per i-block:
  - PE computes psum[i, j] = -2 x_i.x_j + sq_j with K=8 fp16 matmuls
    (hi/lo splits keep ~1e-6 accuracy), 512-wide chunks cycling PE row
    groups 0/32/64/96 via tile_position (operands host-replicated at
    partition offsets 0/32/64/96).
  - The compare d2 < R^2  <=>  psum < thresh_i splits between ACT
    (Sign + accum -> S) and DVE (is_lt + accum -> C) at a per-block
    wa balancing the two engines' measured rates.  Blocks wider than
    the 1536-col psum tile put the overflow in a second tile, scanned
    by DVE into a separate accumulator.
  - Per 8-block chunk, DVE merges counts T = 0.5*(S + wa) + C (exact
    fp16 ints) into columns [pl, pl, T, T] of a staging tile; a PE
    transpose (identity matmul) flips it to [4k, 128] fp16 in PSUM,
    one ACT copy lands it in SBUF as the projection lhsT.  Fully
    on-chip; projection of chunk k overlaps counts of chunk k+1.
  - Projection is one K=4 fp16 matmul per 128 points (rhs rows hi/lo
    of [W0, W1/20], replicated at each 4-partition offset so lhsT/rhs
    partition offsets agree).  Outputs are written fp16 (host
    upcasts and un-permutes the x-sort).
"""

import numpy as np

import concourse.bass as bass
import concourse.mybir as mybir
import concourse.tile as tile
from concourse import bacc, bass_utils

R2 = 0.16
SCALE = 20.0
BIG = 32768.0          # pad sentinel; fp16-exact and >> R2
N_CORES = 8
WCAP = 1536            # scan psum tile width

F32 = mybir.dt.float32
F16 = mybir.dt.float16

# chunk boundaries (i-blocks per transpose); last chunks small to
# shrink the end-of-kernel tail.
CHUNKS = ((0, 8), (8, 16), (16, 24), (24, 28), (28, 32))
PROJ_DELAY = 2         # blocks between chunk end and its proj blocks


def _split16(v):
    hi = v.astype(np.float16)
    lo = (v - hi.astype(np.float32)).astype(np.float16)
    return hi, lo


def _wa_for(W):
    """ACT-side width: balances ACT (236ns + wa/1.2) vs DVE
    (127ns + wd/0.96), measured.  Overflow beyond WCAP goes to DVE."""
    Wm = min(W, WCAP)
    wa = int(round(0.5556 * Wm - 58.0))
    return max(0, min(Wm, wa))


def build_program(N, BPC, Mpad, LO, WID):
    C = N // 128
    assert len(LO) == C and len(WID) == C
    WA = [_wa_for(w) for w in WID]
    has_wide = [w > WCAP for w in WID]
    chunk_wide = [any(has_wide[c0:c1]) for c0, c1 in CHUNKS]

    nc = bacc.Bacc("TRN2", target_bir_lowering=False, debug=False,
                   num_devices=N_CORES)

    xr_d = nc.dram_tensor("xirhs", [BPC, 4, 8, N + Mpad], F16,
                          kind="ExternalInput")
    ax_d = nc.dram_tensor("aux", [BPC, 128, 3 * C], F32,
                          kind="ExternalInput")
    id_d = nc.dram_tensor("ident", [128, 128], F32, kind="ExternalInput")
    rw_d = nc.dram_tensor("rhsW", [16, 8 * 384], F16, kind="ExternalInput")
    pj_d = nc.dram_tensor("proj", [BPC, N, 384], F16, kind="ExternalOutput")

    with tile.TileContext(nc) as tc:
        with (
            tc.tile_pool(name="const", bufs=BPC) as cpool,
            tc.tile_pool(name="accp", bufs=BPC) as accp,
            tc.tile_pool(name="sp", bufs=2, space="PSUM") as spp,
            tc.tile_pool(name="pp", bufs=2, space="PSUM") as ppp,
            tc.tile_pool(name="scr_a", bufs=3) as scra,
            tc.tile_pool(name="scr_d", bufs=3) as scrd,
            tc.tile_pool(name="ct", bufs=2) as ctp,
            tc.tile_pool(name="outsb", bufs=8) as outp,
            tc.tile_pool(name="w", bufs=1) as wpool,
        ):
            dma_engs = (nc.sync, nc.gpsimd)
            load_engs = (nc.sync, nc.gpsimd, nc.scalar)

            xr, aux = [None] * BPC, [None] * BPC
            acc_a, acc_d, acc_w = [], [], []

            def load_batch(b):
                t = cpool.tile([128, 3 * C], F32, tag="ax", name=f"ax{b}")
                load_engs[b % 3].dma_start(t[:], ax_d.ap()[b])
                aux[b] = t
                t = cpool.tile([128, N + Mpad], F16, tag="xr", name=f"xr{b}")
                for g in range(4):
                    load_engs[(g + 1) % 3].dma_start(
                        t[32 * g:32 * g + 8, :], xr_d.ap()[b][g])
                xr[b] = t

            load_batch(0)
            ident = wpool.tile([128, 128], F32, tag="id", name="ident")
            nc.gpsimd.dma_start(ident[:], id_d.ap())
            rhsW = wpool.tile([16, 8 * 384], F16, tag="rw", name="rhsW")
            nc.scalar.dma_start(rhsW[:], rw_d.ap())
            for b in range(BPC):
                acc_a.append(accp.tile([128, C], F32, tag="aa", name=f"aa{b}"))
                acc_d.append(accp.tile([128, C], F32, tag="ad", name=f"ad{b}"))
                t = accp.tile([128, C], F32, tag="aw", name=f"aw{b}")
                nc.gpsimd.memset(t[:], 0.0)
                acc_w.append(t)

            def counts_block(b, ib):
                W, lo, wa = WID[ib], LO[ib], WA[ib]
                Wm = min(W, WCAP)
                isl = slice(ib * 128, (ib + 1) * 128)
                sp = spp.tile([128, WCAP], F32, tag="sp", name=f"sp_{b}_{ib}")
                thb = aux[b][:, ib:ib + 1]
                off = 0
                g = ib  # rotate PE row groups across chunks and blocks
                while off < Wm:
                    w = min(512, Wm - off)
                    grp = 32 * (g % 4)
                    nc.tensor.matmul(
                        sp[:, off:off + w],
                        xr[b][grp:grp + 8, isl],
                        xr[b][grp:grp + 8, N + lo + off:N + lo + off + w],
                        start=True, stop=True,
                        tile_position=(grp, 0))
                    off += 512
                    g += 1
                spx = None
                if W > WCAP:
                    assert W - WCAP <= 512
                    spx = ppp.tile([128, 512], F32, tag="po",
                                   name=f"spx_{b}_{ib}")
                    grp = 32 * (g % 4)
                    nc.tensor.matmul(
                        spx[:, :W - WCAP],
                        xr[b][grp:grp + 8, isl],
                        xr[b][grp:grp + 8, N + lo + WCAP:N + lo + W],
                        start=True, stop=True,
                        tile_position=(grp, 0))
                sa = scra.tile([128, 1024], F16, tag="sa", name=f"sa_{b}_{ib}")
                nc.scalar.activation(
                    sa[:, :wa], sp[:, :wa],
                    mybir.ActivationFunctionType.Sign,
                    bias=thb, scale=-1.0,
                    accum_out=acc_a[b][:, ib:ib + 1])
                sd = scrd.tile([128, 1024], F16, tag="sd", name=f"sd_{b}_{ib}")
                nc.vector.tensor_scalar(
                    sd[:, :Wm - wa], sp[:, wa:Wm],
                    thb, None,
                    op0=mybir.AluOpType.is_lt,
                    op1=mybir.AluOpType.add,
                    accum_out=acc_d[b][:, ib:ib + 1])
                if spx is not None:
                    sx = scrd.tile([128, 1024], F16, tag="sd",
                                   name=f"sx_{b}_{ib}")
                    nc.vector.tensor_scalar(
                        sx[:, :W - WCAP], spx[:, :W - WCAP],
                        thb, None,
                        op0=mybir.AluOpType.is_lt,
                        op1=mybir.AluOpType.add,
                        accum_out=acc_w[b][:, ib:ib + 1])

            lhsT = {}   # (b, chunk) -> sbuf [16, 128] tile of [pl, T] rows
            cts = {}    # (b, chunk) -> staging tile

            def stage_chunk(b, k):
                """Copy pl columns early (no dependence on counts)."""
                c0, c1 = CHUNKS[k]
                ncol = c1 - c0
                ct = ctp.tile([128, 16], F32, tag="ct", name=f"ct_{b}_{k}")
                ctv = ct[:].rearrange("p (c f) -> p c f", f=2)
                nc.vector.tensor_copy(ctv[:, :ncol, 0],
                                      aux[b][:, C + c0:C + c1])
                cts[(b, k)] = ct

            def merge_chunk(b, k):
                c0, c1 = CHUNKS[k]
                n2 = 2 * (c1 - c0)
                ncol = c1 - c0
                ct = cts.pop((b, k))
                ctv = ct[:].rearrange("p (c f) -> p c f", f=2)
                csl = slice(c0, c1)
                # T = 0.5*(S + waC) + C (+ wide overflow), exact fp32
                tmp = accp.tile([128, 8], F32, tag="tmp", name=f"tm_{b}_{k}")
                nc.vector.tensor_tensor(
                    tmp[:, :ncol], acc_a[b][:, csl],
                    aux[b][:, 2 * C + c0:2 * C + c1],
                    op=mybir.AluOpType.add)
                if chunk_wide[k]:
                    nc.vector.tensor_tensor(
                        tmp[:, :ncol], tmp[:, :ncol], acc_w[b][:, csl],
                        op=mybir.AluOpType.add)
                nc.vector.scalar_tensor_tensor(
                    ctv[:, :ncol, 1], tmp[:, :ncol], 0.5, acc_d[b][:, csl],
                    op0=mybir.AluOpType.mult, op1=mybir.AluOpType.add)
                # transpose -> [2k, 128] f32 psum (borrow a proj tile) -> SBUF
                tps = ppp.tile([128, 512], F32, tag="po", name=f"tp_{b}_{k}")
                nc.tensor.transpose(tps[:n2, :128], ct[:, :n2], ident[:])
                lt = ctp.tile([16, 128], F16, tag="lt", name=f"lt_{b}_{k}")
                nc.scalar.copy(lt[:n2, :], tps[:n2, :128])
                lhsT[(b, k)] = lt

            osb_cur = [None]

            def proj_block(b, k, ib):
                c0, c1 = CHUNKS[k]
                q = ib - c0
                kk = 2 * (c1 - c0)
                po = ppp.tile([128, 512], F32, tag="po", name=f"po_{b}_{ib}")
                # lhsT is the whole transposed chunk (32-aligned partition
                # start); rhs variant q is zero outside rows 2q..2q+2, so
                # the K-dim contraction picks out block q's [pl, T].
                nc.tensor.matmul(
                    po[:, :384],
                    lhsT[(b, k)][:kk, :],
                    rhsW[:kk, 384 * q:384 * q + 384], start=True, stop=True,
                    tile_position=(0, 0))
                half = ib % 2
                if half == 0:
                    osb_cur[0] = outp.tile([128, 768], F16, tag="osb",
                                           name=f"osb_{b}_{ib}")
                osb = osb_cur[0]
                dst = osb[:].rearrange("p (s f) -> p s f", s=2)[:, half, :]
                if half == 0:
                    nc.vector.tensor_copy(dst, po[:, :384])
                else:
                    nc.scalar.copy(dst, po[:, :384])
                    src_ = osb[:].rearrange("p (s f) -> p s f", s=2)
                    ddst = pj_d.ap()[b, (ib - 1) * 128:(ib + 1) * 128, :] \
                        .rearrange("(s p) f -> p s f", p=128)
                    dma_engs[(ib // 2) % 2].dma_start(ddst, src_)

            # ---- schedule ----
            pending = []
            ready_at = {}
            for b in range(BPC):
                for ib in range(32):
                    for k, (c0, c1) in enumerate(CHUNKS):
                        if ib == c0:
                            stage_chunk(b, k)
                    counts_block(b, ib)
                    if b + 1 < BPC and ib == 1:
                        load_batch(b + 1)
                    here = b * 32 + ib
                    for _ in range(3):
                        if pending and ready_at[pending[0]] <= here:
                            pb, pk, pib = pending.pop(0)
                            proj_block(pb, pk, pib)
                    for k, (c0, c1) in enumerate(CHUNKS):
                        if ib == c1 - 1:
                            merge_chunk(b, k)
                            for j in range(c0, c1):
                                pending.append((b, k, j))
                                ready_at[(b, k, j)] = here + PROJ_DELAY
            for pb, pk, pib in pending:
                proj_block(pb, pk, pib)
    nc.compile()
    return nc


def _windows(locs, placed):
    """Union (over batches) per-block j-windows into x-sorted placed."""
    B, N, _ = locs.shape
    C = N // 128
    R = np.float32(np.sqrt(R2))
    lo = np.full(C, 1 << 30)
    hi = np.zeros(C, dtype=np.int64)
    for b in range(B):
        x = locs[b, :, 0]
        xs = np.sort(x)
        px = np.sort(x[placed[b]])
        l = np.searchsorted(px, xs[::128] - R, side="left")
        h = np.searchsorted(px, xs[127::128] + R, side="right")
        np.minimum(lo, l, out=lo)
        np.maximum(hi, h, out=hi)
    wid = hi - lo
    return lo.astype(int).tolist(), wid.astype(int).tolist()


def prep_core_inputs(action_mask, keepout, probe, locs, W, Mpad, LO, WID):
    BPC, N, _ = locs.shape
    C = N // 128
    WA_tot = [_wa_for(w) for w in WID]

    placed = (~action_mask) & ~(keepout | probe)
    x = locs.astype(np.float32)
    sq = (x ** 2).sum(-1)

    xirhs = np.zeros((BPC, 8, N + Mpad), np.float16)
    aux = np.zeros((BPC, 128, 3 * C), np.float32)
    orders = []

    for b in range(BPC):
        order = np.argsort(x[b, :, 0], kind="stable")
        orders.append(order)
        xs = x[b, order]
        sqs = sq[b, order]
        pls = placed[b, order].astype(np.float32)

        x0h, x0l = _split16(xs[:, 0])
        x1h, x1l = _split16(xs[:, 1])
        xirhs[b, 0, :N] = x0h
        xirhs[b, 1, :N] = x0h
        xirhs[b, 2, :N] = x0l
        xirhs[b, 3, :N] = x1h
        xirhs[b, 4, :N] = x1h
        xirhs[b, 5, :N] = x1l
        xirhs[b, 6, :N] = 1.0
        xirhs[b, 7, :N] = 1.0

        pidx = np.nonzero(placed[b])[0]
        psort = pidx[np.argsort(x[b, pidx, 0], kind="stable")]
        np_ = len(psort)
        assert np_ <= Mpad
        j0h, j0l = _split16(-2.0 * x[b, psort, 0])
        j1h, j1l = _split16(-2.0 * x[b, psort, 1])
        sqh, sql = _split16(sq[b, psort])
        r = xirhs[b, :, N:]
        r[6, :] = BIG
        r[0, :np_] = j0h
        r[1, :np_] = j0l
        r[2, :np_] = j0h
        r[3, :np_] = j1h
        r[4, :np_] = j1l
        r[5, :np_] = j1h
        r[6, :np_] = sqh
        r[7, :np_] = sql

        aux[b, :, 0:C] = (R2 - sqs).reshape(C, 128).T
        aux[b, :, C:2 * C] = pls.reshape(C, 128).T
        aux[b, :, 2 * C:] = np.asarray(WA_tot, np.float32)[None, :]

    W = W.astype(np.float32)
    rhsW = np.zeros((16, 8 * 384), np.float16)
    for q in range(8):
        rhsW[2 * q, 384 * q:384 * (q + 1)] = W[0].astype(np.float16)
        rhsW[2 * q + 1, 384 * q:384 * (q + 1)] = (W[1] / SCALE).astype(
            np.float16)

    ident = np.eye(128, dtype=np.float32)
    rep4 = lambda a: np.repeat(a[:, None], 4, axis=1)
    return {
        "xirhs": rep4(xirhs), "aux": aux, "rhsW": rhsW, "ident": ident,
    }, orders


_PROGRAM_CACHE = {}


def kernel(action_mask, keepout, probe, locs, W, _trace=False, _tmpdir=None):
    action_mask = np.asarray(action_mask)
    keepout = np.asarray(keepout)
    probe = np.asarray(probe)
    locs = np.asarray(locs, dtype=np.float32)
    W = np.asarray(W, dtype=np.float32)

    B, N = action_mask.shape
    BPC = B // N_CORES

    placed = (~action_mask) & ~(keepout | probe)
    max_placed = int(placed.sum(1).max())
    Mpad = ((max_placed + 63) // 64) * 64
    LO, WID = _windows(locs, placed)

    key = (N, BPC, Mpad, tuple(LO), tuple(WID))
    if key not in _PROGRAM_CACHE:
        _PROGRAM_CACHE[key] = build_program(N, BPC, Mpad, LO, WID)
    nc = _PROGRAM_CACHE[key]

    in_maps = []
    orders = []
    for c in range(N_CORES):
        s = slice(c * BPC, (c + 1) * BPC)
        m, o = prep_core_inputs(
            action_mask[s], keepout[s], probe[s], locs[s], W, Mpad, LO, WID)
        in_maps.append(m)
        orders.extend(o)

    res = bass_utils.run_bass_kernel_spmd(
        nc, in_maps, core_ids=list(range(N_CORES)),
        trace=_trace, tmpdir=_tmpdir)

    proj_s = np.concatenate(
        [res.results[c]["proj"] for c in range(N_CORES)], 0)
    proj = np.empty((B, N, 384), np.float32)
    for b in range(B):
        proj[b, orders[b]] = proj_s[b].astype(np.float32)
    out = (np.ascontiguousarray(proj[:, :, :128]),
           np.ascontiguousarray(proj[:, :, 128:256]),
           np.ascontiguousarray(proj[:, :, 256:384]))
    if _trace:
        return out, res
    return out
